# revision 47
# baseline (speedup 1.0000x reference)
"""AudioMamba2 fused TRN2 kernel: 8-core data-parallel Bass/Tile.

Self-contained: host folds weights, transposes x to bf16 xT[37,N] with a
baked ones row, runs a two-phase (silu / exp-ln table set) row-major
pipeline per core, returns the full [N, 32] softmax output.
"""
import numpy as np
import ml_dtypes
from contextlib import ExitStack

import concourse.bass as bass
import concourse.mybir as mybir
import concourse.tile as tile
from concourse.bass_types import AP

F32 = mybir.dt.float32
BF16 = mybir.dt.bfloat16
AF = mybir.ActivationFunctionType
ALU = mybir.AluOpType

IN_DIM = 36
D_MODEL = 32
D_INNER = 64
D_STATE = 8
NHEADS = 8
HEADDIM = 8
CONV_DIM = 80
D_IN_PROJ = 152
NORM_EPS = 1e-5
K1 = 37          # 36 features + ones row
NSIL = 144       # z|xh|B|C channels (silu'd)


def fold_weights(f_out_w, f_out_b, in_proj_w, conv_w, conv_b, dt_bias,
                 A_log, D_skip, norm_w, out_proj_w):
    f64 = np.float64
    W12 = in_proj_w.astype(f64) @ f_out_w.astype(f64)          # [152, 36]
    b12 = in_proj_w.astype(f64) @ f_out_b.astype(f64)          # [152]
    s80 = conv_w[:, -1].astype(f64)
    W12[64:144] *= s80[:, None]
    b12[64:144] = b12[64:144] * s80 + conv_b.astype(f64)
    b12[144:152] += dt_bias.astype(f64)
    W1 = np.concatenate([W12, b12[:, None]], axis=1)           # [152, 37]
    W1T = np.ascontiguousarray(W1.T)                           # [37, 152]
    Wout = out_proj_w.astype(f64) * norm_w.astype(f64)[None, :]  # [32, 64]
    WoutT = np.ascontiguousarray(Wout.T)                       # [64, 32]
    WoutT2 = np.concatenate([WoutT, WoutT], axis=0)            # [128, 32]
    return (W1T.astype(ml_dtypes.bfloat16),
            WoutT2.astype(ml_dtypes.bfloat16),
            np.ascontiguousarray(
                np.broadcast_to(D_skip.astype(np.float32), (128, 8))))


def prep_xt(x):
    """x [N, 36] f32 -> xT [37, N] bf16 with ones row."""
    N = x.shape[0]
    xt = np.empty((K1, N), dtype=ml_dtypes.bfloat16)
    xt[:IN_DIM] = x.T.astype(ml_dtypes.bfloat16)
    xt[IN_DIM] = np.float32(1.0)
    return xt


def bcast(ap, count):
    """Append a step-0 innermost free dim of size `count` to an AP."""
    return AP(ap.tensor, ap.offset, list(ap.ap) + [[0, count]])


def build_kernel(npc, num_cores=8, y_engine="vector", sim_safe=False,
                 debug_stop=None):
    """Build the Bass program for one core processing npc rows."""
    assert npc % 128 == 0
    NB = npc // 128
    nc = bass.Bass("TRN2", target_bir_lowering=False, num_devices=num_cores)

    xt_d = nc.dram_tensor("xt", [K1, npc], BF16, kind="ExternalInput")
    w1t_d = nc.dram_tensor("w1t", [K1, D_IN_PROJ], BF16, kind="ExternalInput")
    woutt_d = nc.dram_tensor("woutt", [128, D_MODEL], BF16,
                             kind="ExternalInput")
    db_d = nc.dram_tensor("db", [128, NHEADS], F32, kind="ExternalInput")
    eps_d = nc.dram_tensor("eps", [128, 1], F32, kind="ExternalInput")
    id_d = nc.dram_tensor("ident", [128, 128], BF16, kind="ExternalInput")
    out_d = nc.dram_tensor("out", [npc, D_MODEL], F32, kind="ExternalOutput")

    # persistent SBUF stores
    w1t_s = nc.alloc_sbuf_tensor("w1t_s", [K1, D_IN_PROJ], BF16)
    woutt_s = nc.alloc_sbuf_tensor("woutt_s", [128, D_MODEL], BF16)
    db_s = nc.alloc_sbuf_tensor("db_s", [128, NHEADS], F32)
    eps_s = nc.alloc_sbuf_tensor("eps_s", [128, 1], F32)
    id_s = nc.alloc_sbuf_tensor("id_s", [128, 128], BF16)
    t1_st = nc.alloc_sbuf_tensor("t1_st", [128, NB, D_INNER], BF16)
    w_st = nc.alloc_sbuf_tensor("w_st", [128, NB, NHEADS], F32)
    bc_st = nc.alloc_sbuf_tensor("bc_st", [128, NB], F32)
    ss_st = nc.alloc_sbuf_tensor("ss_st", [128, NB], F32)
    r_st = nc.alloc_sbuf_tensor("r_st", [128, NB], F32)

    XCH = 64            # x-in DMA chunk, blocks
    GP = 3              # PSUM silu-group
    GW = 24             # w (dt-preact) PSUM group (multiple of GP)
    GB = 16             # DVE batch group (beta)
    GD = 16             # dt/r batch group
    GO = 16             # out2/exp group

    ve = nc.vector
    ye = {"vector": nc.vector, "gpsimd": nc.gpsimd}[y_engine]

    with tile.TileContext(nc) as tc:
        # one-time const loads
        nc.sync.dma_start(w1t_s.ap(), w1t_d.ap())
        nc.sync.dma_start(woutt_s.ap(), woutt_d.ap())
        nc.sync.dma_start(db_s.ap(), db_d.ap())
        nc.sync.dma_start(eps_s.ap(), eps_d.ap())
        nc.sync.dma_start(id_s.ap(), id_d.ap())

        with (
            tc.tile_pool(name="xtp", bufs=3) as xtp,
            tc.tile_pool(name="pa", bufs=5, space="PSUM") as pap,
            tc.tile_pool(name="wps", bufs=2, space="PSUM") as wpsp,
            tc.tile_pool(name="sp", bufs=6) as sp,
            tc.tile_pool(name="prp", bufs=3) as prp,
        ):
            # ---------------- phase A ----------------
            xt_tiles = {}
            w_ps = None
            blocks = list(range(NB))
            groups = [blocks[i:i + GP] for i in range(0, NB, GP)]
            for grp in groups:
                g0 = grp[0]
                for b in grp:
                    ci = b // XCH
                    if ci not in xt_tiles:
                        t = xtp.tile([K1, XCH * 128], BF16)
                        c0 = ci * XCH
                        nc.sync.dma_start(
                            t[:, : min(XCH, NB - c0) * 128],
                            xt_d[:, c0 * 128: min(c0 + XCH, NB) * 128])
                        xt_tiles[ci] = t
                if g0 % GW == 0:
                    w_ps = wpsp.tile([128, GW * NHEADS], F32)
                ng = len(grp)
                P = pap.tile([128, GP * NSIL], F32)
                for j, b in enumerate(grp):
                    xt_sl = xt_tiles[b // XCH][
                        :, (b % XCH) * 128: (b % XCH) * 128 + 128]
                    nc.tensor.matmul(P[:, j * NSIL:(j + 1) * NSIL],
                                     xt_sl, w1t_s[:, 0:NSIL])
                    nc.tensor.matmul(
                        w_ps[:, (b % GW) * NHEADS:(b % GW + 1) * NHEADS],
                        xt_sl, w1t_s[:, NSIL:D_IN_PROJ])
                S = sp.tile([128, GP, NSIL], BF16)
                Pv = P.rearrange("p (g c) -> p g c", c=NSIL)[:, :ng, :]
                if sim_safe:
                    # CoreSim lacks Silu: sigmoid + explicit mul
                    nc.scalar.activation(S[:, :ng, :], Pv, AF.Sigmoid)
                    ve.tensor_tensor(out=S[:, :ng, :], in0=S[:, :ng, :],
                                     in1=Pv, op=ALU.mult)
                else:
                    nc.scalar.activation(S[:, :ng, :], Pv, AF.Silu)
                # t1 = S_z * S_xh
                ve.tensor_tensor(
                    out=t1_st[:, g0:g0 + ng, :],
                    in0=S[:, :ng, 0:64], in1=S[:, :ng, 64:128],
                    op=ALU.mult)
                # bc = sum(S_B * S_C)
                pr = prp.tile([128, GP, D_STATE], BF16)
                ve.tensor_tensor(out=pr[:, :ng, :],
                                 in0=S[:, :ng, 128:136], in1=S[:, :ng, 136:144],
                                 op=ALU.mult)
                ve.tensor_reduce(out=bc_st[:, g0:g0 + ng], in_=pr[:, :ng, :],
                                 axis=mybir.AxisListType.X, op=ALU.add)
                if (g0 + ng) % GW == 0 or (g0 + ng) == NB:
                    wg0 = (g0 + ng - 1) // GW * GW
                    nw = g0 + ng - wg0
                    nc.scalar.activation(
                        w_st[:, wg0:wg0 + nw, :],
                        w_ps.rearrange("p (g c) -> p g c", c=NHEADS)[:, :nw, :],
                        AF.Copy)

        if debug_stop == "a":
            with tc.tile_pool(name="zp", bufs=1) as zp:
                z = zp.tile([128, NB, D_MODEL], F32)
                nc.vector.memset(z, 0.0)
                nc.sync.dma_start(
                    out_d.rearrange("(nb p) c -> p nb c", p=128), z)
            return nc

        # ---------------- phase B ----------------
        with (
            tc.tile_pool(name="dtp", bufs=3) as dtp,
            tc.tile_pool(name="fp", bufs=3) as fp,
            tc.tile_pool(name="yp", bufs=3) as yp,
            tc.tile_pool(name="ytpp", bufs=3, space="PSUM") as ytpp,
            tc.tile_pool(name="ytp", bufs=4) as ytp,
            tc.tile_pool(name="sqp", bufs=2) as sqp,
            tc.tile_pool(name="o2p", bufs=3, space="PSUM") as o2p,
            tc.tile_pool(name="onp", bufs=2) as onp,
            tc.tile_pool(name="ep", bufs=3) as ep,
            tc.tile_pool(name="sep", bufs=2) as sep,
            tc.tile_pool(name="osp", bufs=3) as osp,
        ):
            zp_ctx = None
            for m0 in range(0, NB, GD):     # 16-block macro
                nm = min(GD, NB - m0)
                # softplus: dt = ln(1 + exp(w))
                dt_t = dtp.tile([128, GD, NHEADS], F32)
                nc.scalar.activation(dt_t[:, :nm, :], w_st[:, m0:m0 + nm, :],
                                     AF.Exp)
                nc.scalar.activation(dt_t[:, :nm, :], dt_t[:, :nm, :],
                                     AF.Ln, bias=1.0)
                yt_tiles = []
                for q0 in range(m0, m0 + nm, GB):
                    nq = min(GB, NB - q0)
                    f4 = fp.tile([128, GB, NHEADS], F32)
                    # dtbc = dt * bc_b ; f4 = dtbc + D_b
                    ve.tensor_tensor(
                        out=f4[:, :nq, :],
                        in0=dt_t[:, q0 - m0:q0 - m0 + nq, :],
                        in1=bcast(bc_st[:, q0:q0 + nq], NHEADS),
                        op=ALU.mult)
                    ve.tensor_tensor(
                        out=f4[:, :nq, :], in0=f4[:, :nq, :],
                        in1=AP(db_s.ap().tensor, 0,
                               [[NHEADS, 128], [0, GB], [1, NHEADS]])[:, :nq, :],
                        op=ALU.add)
                    # y_u = t1 * f4_b   (bf16, pair layout for xbar)
                    yu = yp.tile([128, GB * D_INNER], BF16)
                    ye.tensor_tensor(
                        out=yu.rearrange("p (g c) -> p g c", c=D_INNER)[:, :nq, :],
                        in0=t1_st[:, q0:q0 + nq, :]
                            .rearrange("p g (h d) -> p g h d", d=HEADDIM),
                        in1=bcast(f4[:, :nq, :], HEADDIM),
                        op=ALU.mult)
                    # transpose pairs -> yT (PE transpose + PSUM->SBUF copy)
                    for pi in (range(0, nq, 2) if debug_stop not in ("b1",) else []):
                        ytps = ytpp.tile([128, 128], BF16)
                        nc.tensor.transpose(ytps, yu[:, pi * 64:(pi + 2) * 64],
                                            id_s.ap())
                        ytt = ytp.tile([128, 128], BF16)
                        if (pi // 2) % 2 == 0:
                            nc.scalar.copy(ytt, ytps)
                        else:
                            ve.tensor_copy(ytt, ytps)
                        yt_tiles.append(ytt)
                    # ss = sum(y_u^2)
                    sq = sqp.tile([128, GB, D_INNER], BF16)
                    ve.tensor_tensor(
                        out=sq[:, :nq, :],
                        in0=yu.rearrange("p (g c) -> p g c", c=D_INNER)[:, :nq, :],
                        in1=yu.rearrange("p (g c) -> p g c", c=D_INNER)[:, :nq, :],
                        op=ALU.mult)
                    ve.tensor_reduce(out=ss_st[:, q0:q0 + nq], in_=sq[:, :nq, :],
                                     axis=mybir.AxisListType.X, op=ALU.add)
                # r = (ss/64 + eps)^-1/2 = exp(-0.5*ln(ss/64 + eps))
                nc.scalar.activation(r_st[:, m0:m0 + nm], ss_st[:, m0:m0 + nm],
                                     AF.Ln, bias=eps_s.ap(), scale=1.0 / 64)
                nc.scalar.activation(r_st[:, m0:m0 + nm], r_st[:, m0:m0 + nm],
                                     AF.Exp, scale=-0.5)
                # MM2 + softmax per GO-group
                for h0 in (range(m0, m0 + nm, GO) if debug_stop not in ("b1", "b1x") else []):
                    nh = min(GO, NB - h0)
                    assert nh % 2 == 0
                    GOH = GO // 2
                    for par in range(2):       # 0: even blocks, 1: odd
                        nhp = nh // 2
                        o2 = o2p.tile([128, GOH * D_MODEL], F32)
                        for j in range(nhp):
                            b = h0 + 2 * j + par
                            ytt = yt_tiles[(b - m0) // 2]
                            lhs = ytt[par * 64:par * 64 + 64, :]
                            rhs_w = woutt_s[par * 64:par * 64 + 64, :]
                            nc.tensor.matmul(
                                o2[:, j * D_MODEL:(j + 1) * D_MODEL],
                                lhs, rhs_w)
                        blk_sel = slice(h0 + par, h0 + nh, 2)
                        on = onp.tile([128, GOH, D_MODEL], F32)
                        ve.tensor_tensor(
                            out=on[:, :nhp, :],
                            in0=o2.rearrange("p (g c) -> p g c",
                                             c=D_MODEL)[:, :nhp, :],
                            in1=bcast(r_st[:, blk_sel], D_MODEL),
                            op=ALU.mult)
                        e_t = ep.tile([128, GOH, D_MODEL], F32)
                        nc.scalar.activation(e_t[:, :nhp, :], on[:, :nhp, :],
                                             AF.Exp)
                        se = sep.tile([128, GOH], F32)
                        ve.tensor_reduce(out=se[:, :nhp], in_=e_t[:, :nhp, :],
                                         axis=mybir.AxisListType.X, op=ALU.add)
                        rec = sep.tile([128, GOH], F32)
                        ve.reciprocal(rec[:, :nhp], se[:, :nhp])
                        os_t = osp.tile([128, GOH, D_MODEL], F32)
                        ve.tensor_tensor(out=os_t[:, :nhp, :],
                                         in0=e_t[:, :nhp, :],
                                         in1=bcast(rec[:, :nhp], D_MODEL),
                                         op=ALU.mult)
                        nc.sync.dma_start(
                            out_d.rearrange("(nb p) c -> p nb c", p=128)
                                 [:, blk_sel, :],
                            os_t[:, :nhp, :])
    if debug_stop in ("b1", "b1x"):
        with tile.TileContext(nc) as tc2:
            with tc2.tile_pool(name="zp2", bufs=1) as zp:
                z = zp.tile([128, NB, D_MODEL], F32)
                nc.vector.memset(z, 0.0)
                nc.sync.dma_start(
                    out_d.rearrange("(nb p) c -> p nb c", p=128), z)
    return nc


CTRL_OPS = ("Drain", "NoOp", "Nop", "EventSemaphoreOp", "SemaphoreOp")


def split_overloaded_waits(nc, cap=1, ctrl_only=False):
    n_fixed = 0
    for f in nc.m.functions:
        for bb in f.blocks:
            insts = bb.instructions
            i = 0
            while i < len(insts):
                ins = insts[i]
                si = ins.sync_info
                eff_cap = cap
                if ctrl_only and str(ins.opcode) not in CTRL_OPS:
                    eff_cap = 255
                if si is not None and si.on_wait and len(si.on_wait) > eff_cap:
                    waits = list(si.on_wait)
                    extra, keep = waits[:-cap], waits[-cap:]
                    pos = i
                    for j in range(0, len(extra), cap):
                        chunk = extra[j:j + cap]  # noqa
                        nop = mybir.InstNoOp(
                            name=nc.get_next_instruction_name(), ins=[], outs=[])
                        nop.engine = ins.engine
                        nop.sync_info = mybir.SyncInfo(on_wait=chunk,
                                                       on_update=[])
                        nc.register_instruction(nop)
                        insts.insert(pos, nop)
                        pos += 1
                        i += 1
                    si.on_wait = keep
                    ins.sync_info = si
                    n_fixed += 1
                i += 1
    return n_fixed


def run(x, f_out_w, f_out_b, in_proj_w, conv_w, conv_b, dt_bias, A_log,
        D_skip, norm_w, out_proj_w, num_cores=8, y_engine="vector",
        trace=False, sim_safe=False):
    from concourse.bass_utils import run_bass_kernel_spmd
    N = x.shape[0]
    assert N % (num_cores * 128) == 0
    npc = N // num_cores
    w1t, woutt, db = fold_weights(f_out_w, f_out_b, in_proj_w, conv_w,
                                  conv_b, dt_bias, A_log, D_skip, norm_w,
                                  out_proj_w)
    xt = prep_xt(x)
    nc = build_kernel(npc, num_cores=num_cores, y_engine=y_engine,
                      sim_safe=sim_safe)
    split_overloaded_waits(nc)
    in_maps = []
    for c in range(num_cores):
        in_maps.append({
            "xt": np.ascontiguousarray(xt[:, c * npc:(c + 1) * npc]),
            "w1t": w1t, "woutt": woutt, "db": db,
            "eps": np.full((128, 1), NORM_EPS, np.float32),
            "ident": np.eye(128, dtype=ml_dtypes.bfloat16),
        })
    res = run_bass_kernel_spmd(nc, in_maps, list(range(num_cores)),
                               trace=trace)
    out = np.concatenate([res.results[c]["out"] for c in range(num_cores)],
                         axis=0)
    return out, res


_CACHED = {}


def kernel(x, f_out_w, f_out_b, in_proj_w, conv_w, conv_b, dt_bias, A_log,
           D_skip, norm_w, out_proj_w):
    out, _ = run(x, f_out_w, f_out_b, in_proj_w, conv_w, conv_b, dt_bias,
                 A_log, D_skip, norm_w, out_proj_w, num_cores=8,
                 y_engine="gpsimd" if _CACHED.get("ye") != "vector"
                 else "vector")
    return out.astype(np.float32)



# revision 48
# speedup vs baseline: 1.1952x; 1.1952x over previous
"""AudioMamba2 fused TRN2 kernel: 8-core data-parallel Bass/Tile.

Two-phase design (16-row-block macros):
  phase 1: MM1 (x @ W1, PE, ping-pong PSUM halves) -> silu (Act) ->
           t1 / B*C / f4 (DVE) -> y = t1*f4 (Pool) -> yT via PE
           transposes + PSUM->SBUF copies into a persistent buffer.
  phase 2: sq = yT^2 (DVE) -> ss via selector-mask matmuls (PE) ->
           rsqrt (Act Ln/Exp) -> MM2 (PE) -> logits scale (DVE) ->
           softmax exp (Act) -> sum (DVE) -> normalize (DVE) -> store.
The phase-2 o2 PSUM bank doubles as phase-1 transpose scratch.
"""
import numpy as np
import ml_dtypes
from contextlib import ExitStack

import concourse.bass as bass
import concourse.mybir as mybir
import concourse.tile as tile
from concourse.bass_types import AP

F32 = mybir.dt.float32
BF16 = mybir.dt.bfloat16
AF = mybir.ActivationFunctionType
ALU = mybir.AluOpType

IN_DIM = 36
D_MODEL = 32
D_INNER = 64
D_STATE = 8
NHEADS = 8
HEADDIM = 8
CONV_DIM = 80
D_IN_PROJ = 152
NORM_EPS = 1e-5
K1 = 37          # 36 features + ones row
GB = 16          # macro size in 128-row blocks
XCH = 64         # input DMA chunk, blocks

OPLOG = []


def fold_weights(f_out_w, f_out_b, in_proj_w, conv_w, conv_b, dt_bias,
                 A_log, D_skip, norm_w, out_proj_w):
    f64 = np.float64
    W12 = in_proj_w.astype(f64) @ f_out_w.astype(f64)          # [152, 36]
    b12 = in_proj_w.astype(f64) @ f_out_b.astype(f64)          # [152]
    s80 = conv_w[:, -1].astype(f64)
    W12[64:144] *= s80[:, None]
    b12[64:144] = b12[64:144] * s80 + conv_b.astype(f64)
    b12[144:152] += dt_bias.astype(f64)
    W1 = np.concatenate([W12, b12[:, None]], axis=1)           # [152, 37]
    W1T = np.ascontiguousarray(W1.T)                           # [37, 152]
    Wout = out_proj_w.astype(f64) * norm_w.astype(f64)[None, :]  # [32, 64]
    WoutT = np.ascontiguousarray(Wout.T)                       # [64, 32]
    WoutT2 = np.concatenate([WoutT, WoutT], axis=0)            # [128, 32]
    return (W1T.astype(ml_dtypes.bfloat16),
            WoutT2.astype(ml_dtypes.bfloat16),
            np.ascontiguousarray(
                np.broadcast_to(D_skip.astype(np.float32), (128, 8))))


def prep_xt(x):
    """x [N, 36] f32 -> xT [37, N] bf16 with ones row."""
    N = x.shape[0]
    xt = np.empty((K1, N), dtype=ml_dtypes.bfloat16)
    xt[:IN_DIM] = x.T.astype(ml_dtypes.bfloat16)
    xt[IN_DIM] = np.float32(1.0)
    return xt


def bcast(ap, count):
    """Append a step-0 innermost free dim of size `count` to an AP."""
    return AP(ap.tensor, ap.offset, list(ap.ap) + [[0, count]])


def build_kernel(npc, num_cores=8, sim_safe=False, only=None):
    """Build the Bass program for one core processing npc rows."""
    NB = npc // 128
    assert NB % GB == 0
    NM = NB // GB
    nc = bass.Bass("TRN2", target_bir_lowering=False, num_devices=num_cores)

    xt_d = nc.dram_tensor("xt", [K1, npc], BF16, kind="ExternalInput")
    w1t_d = nc.dram_tensor("w1t", [K1, D_IN_PROJ], BF16, kind="ExternalInput")
    woutt_d = nc.dram_tensor("woutt", [128, D_MODEL], BF16,
                             kind="ExternalInput")
    woutbd_d = nc.dram_tensor("woutbd", [128, 64], BF16,
                              kind="ExternalInput")
    db_d = nc.dram_tensor("db", [128, NHEADS], F32, kind="ExternalInput")
    epsb_d = nc.dram_tensor("epsb", [128, 1], F32, kind="ExternalInput")
    id_d = nc.dram_tensor("ident", [128, 128], BF16, kind="ExternalInput")
    out_d = nc.dram_tensor("out", [npc, D_MODEL], F32, kind="ExternalOutput")

    # persistent SBUF consts + yT store
    w1t_s = nc.alloc_sbuf_tensor("w1t_s", [K1, D_IN_PROJ], BF16)
    woutt_s = nc.alloc_sbuf_tensor("woutt_s", [128, D_MODEL], BF16)
    woutbd_s = nc.alloc_sbuf_tensor("woutbd_s", [128, 64], BF16)
    db_s = nc.alloc_sbuf_tensor("db_s", [128, NHEADS], F32)
    epsb_s = nc.alloc_sbuf_tensor("epsb_s", [128, 1], F32)
    id_s = nc.alloc_sbuf_tensor("id_s", [128, 128], BF16)
    yt_st = nc.alloc_sbuf_tensor("yt_st", [128, NB // 2, 128], BF16)
    ss_st = nc.alloc_sbuf_tensor("ss_st", [128, NB], F32)

    # PSUM map (8 banks): pzx 4 | pbcd 2 | o2/ytp 1 | ssq 1.
    # o2 serves as phase-1 transpose scratch (bf16 view) and phase-2 MM2 out.
    pzx = nc.alloc_psum_tensor("pzx", [128, 2, 8 * 128], F32)
    pbcd = nc.alloc_psum_tensor("pbcd", [128, GB * 32], F32)
    o2 = nc.alloc_psum_tensor("o2", [128, GB * D_MODEL], F32)
    scr0 = nc.alloc_psum_tensor("scr0", [128, 4, 128], BF16)
    scr1 = nc.alloc_psum_tensor("scr1", [128, 4, 128], BF16)

    ve, sc, gp, te, sy = nc.vector, nc.scalar, nc.gpsimd, nc.tensor, nc.sync

    db_bc = AP(db_s.ap().tensor, 0, [[NHEADS, 128], [0, GB], [1, NHEADS]])


    with tile.TileContext(nc) as tc:
        sy.dma_start(w1t_s.ap(), w1t_d.ap())
        sy.dma_start(woutt_s.ap(), woutt_d.ap())
        sy.dma_start(woutbd_s.ap(), woutbd_d.ap())
        sy.dma_start(db_s.ap(), db_d.ap())
        sy.dma_start(epsb_s.ap(), epsb_d.ap())
        sy.dma_start(id_s.ap(), id_d.ap())

        with (
            tc.tile_pool(name="xtp", bufs=3) as xtp,
            tc.tile_pool(name="szxp", bufs=4) as szxp,
            tc.tile_pool(name="sbcp", bufs=3) as sbcp,
            tc.tile_pool(name="dtp", bufs=3) as dtp,
            tc.tile_pool(name="t1p", bufs=4) as t1p,
            tc.tile_pool(name="prp", bufs=3) as prp,
            tc.tile_pool(name="bcp", bufs=3) as bcp,
            tc.tile_pool(name="f4p", bufs=6) as f4p,
            tc.tile_pool(name="yp", bufs=4) as yp,
            tc.tile_pool(name="sqp", bufs=3) as sqp,
            tc.tile_pool(name="rqp", bufs=4) as rqp,
            tc.tile_pool(name="onp", bufs=4) as onp,
            tc.tile_pool(name="ep", bufs=4) as ep,
            tc.tile_pool(name="sep", bufs=4) as sep,
            tc.tile_pool(name="osp", bufs=4) as osp,
        ):
            xt_tiles = {}
            st = {}

            def xtsl(b):
                t = xt_tiles[b // XCH]
                o = (b % XCH) * 128
                return t[:, o:o + 128]

            def load_chunk(ci):
                if ci in xt_tiles or ci * XCH >= NB:
                    return
                t = xtp.tile([K1, XCH * 128], BF16)
                c0 = ci * XCH
                w = min(XCH, NB - c0)
                sy.dma_start(t[:, :w * 128],
                             xt_d[:, c0 * 128:(c0 + w) * 128])
                xt_tiles[ci] = t

            NH = 2 * NM
            mm1_done = set()

            def mm1_half(h):
                """MM1 z|xh for 8-block half h into pzx[h % 2]."""
                if h in mm1_done or h >= NH:
                    return
                mm1_done.add(h)
                for j in range(8):
                    te.matmul(pzx[:, h % 2, j * 128:(j + 1) * 128],
                              xtsl(h * 8 + j), w1t_s[:, 0:128])

            def silu_half(h, szx):
                dst = szx[:, (h % 2) * 8:(h % 2) * 8 + 8, :]
                pz2 = pzx.ap()[:, h % 2, :].rearrange("p (g c) -> p g c",
                                                      c=128)
                if sim_safe:
                    sc.activation(dst, pz2, AF.Sigmoid)
                    ve.tensor_tensor(out=dst, in0=dst, in1=pz2, op=ALU.mult)
                else:
                    sc.activation(dst, pz2, AF.Silu)

            def bcd_mm(k):
                """MM1 for the B|C|dt columns of macro k."""
                if k >= NM:
                    return
                for j in range(GB):
                    te.matmul(pbcd[:, j * 32:j * 32 + 24],
                              xtsl(k * GB + j), w1t_s[:, 128:152])

            def bcd_act(k):
                """silu(B|C) + softplus for macro k."""
                if k >= NM:
                    return
                pb3 = pbcd.ap().rearrange("p (g c) -> p g c", c=32)
                sbc = sbcp.tile([128, GB, 16], BF16)
                if sim_safe:
                    sc.activation(sbc[:, :, :], pb3[:, :, 0:16], AF.Sigmoid)
                    ve.tensor_tensor(out=sbc[:, :, :], in0=sbc[:, :, :],
                                     in1=pb3[:, :, 0:16], op=ALU.mult)
                else:
                    sc.activation(sbc[:, :, :], pb3[:, :, 0:16], AF.Silu)
                dte = dtp.tile([128, GB, NHEADS], F32)
                sc.activation(dte[:, :, :], pb3[:, :, 16:24], AF.Exp)
                sc.activation(dte[:, :, :], dte[:, :, :], AF.Ln, bias=1.0)
                st[("sbc", k)] = sbc
                st[("dt", k)] = dte

            def ytrans(m):
                """PE transposes of y(m) into rotating halves of the o2
                bank, then PSUM->SBUF copies (alternating DVE/Pool)."""
                y = st.pop(("y", m))
                for hh in range(2):
                    scr = [scr0.ap(), scr1.ap()][(2 * m + hh) % 2]
                    for i in range(4):
                        nc.tensor.transpose(
                            scr[:, i, :],
                            y[:, (4 * hh + i) * 128:(4 * hh + i + 1) * 128],
                            id_s.ap())
                    dst = yt_st.ap()[:, m * 8 + 4 * hh:m * 8 + 4 * hh + 4, :]
                    if hh == 0:
                        ve.tensor_copy(dst, scr)
                    else:
                        sc.copy(dst, scr)

            # ---------------- phase 1 ----------------
            for m in range(NM):
                load_chunk((m * GB) // XCH)
                load_chunk((m * GB) // XCH + 1)
                if m >= 1:
                    ytrans(m - 1)
                if m == 0:
                    bcd_mm(0)
                    bcd_act(0)
                    mm1_half(0)
                    mm1_half(1)
                bcd_mm(m + 1)
                szx = szxp.tile([128, GB, 128], BF16)
                t1 = t1p.tile([128, GB, D_INNER], BF16)
                silu_half(2 * m, szx)
                ve.tensor_tensor(out=t1[:, 0:8, :],
                                 in0=szx[:, 0:8, 0:64],
                                 in1=szx[:, 0:8, 64:128], op=ALU.mult)
                mm1_half(2 * m + 2)
                silu_half(2 * m + 1, szx)
                ve.tensor_tensor(out=t1[:, 8:16, :],
                                 in0=szx[:, 8:16, 0:64],
                                 in1=szx[:, 8:16, 64:128], op=ALU.mult)
                mm1_half(2 * m + 3)
                sbc = st.pop(("sbc", m))
                dte = st.pop(("dt", m))
                pr = prp.tile([128, GB, NHEADS], BF16)
                ve.tensor_tensor(out=pr[:, :, :], in0=sbc[:, :, 0:8],
                                 in1=sbc[:, :, 8:16], op=ALU.mult)
                bc = bcp.tile([128, GB], F32)
                ve.tensor_reduce(out=bc[:, :], in_=pr[:, :, :],
                                 axis=mybir.AxisListType.X, op=ALU.add)
                f4 = f4p.tile([128, GB, NHEADS], BF16)
                f4f = f4p.tile([128, GB, NHEADS], F32)
                gp.tensor_tensor(out=f4f[:, :, :], in0=dte[:, :, :],
                                 in1=bcast(bc[:, :], NHEADS), op=ALU.mult)
                ve.tensor_tensor(out=f4[:, :, :], in0=f4f[:, :, :],
                                 in1=db_bc, op=ALU.add)
                y = yp.tile([128, GB * D_INNER], BF16)
                gp.tensor_tensor(
                    out=y.rearrange("p (g h d) -> p g h d", h=NHEADS,
                                    d=HEADDIM),
                    in0=t1.rearrange("p g (h d) -> p g h d", d=HEADDIM),
                    in1=bcast(f4[:, :, :], HEADDIM), op=ALU.mult)
                sqr = sqp.tile([128, GB, D_INNER], BF16)
                ve.tensor_tensor(
                    out=sqr.rearrange("p g c -> p (g c)"), in0=y[:, :],
                    in1=y[:, :], op=ALU.mult)
                ve.tensor_reduce(out=ss_st[:, m * GB:(m + 1) * GB],
                                 in_=sqr[:, :, :],
                                 axis=mybir.AxisListType.X, op=ALU.add)
                st[("y", m)] = y
                bcd_act(m + 1)

            ytrans(NM - 1)

            # ---------------- phase 2 ----------------
            def stage_a(k):
                r = rqp.tile([128, GB], F32)
                sc.activation(r[:, :], ss_st[:, k * GB:(k + 1) * GB],
                              AF.Ln, bias=epsb_s.ap(), scale=1.0 / 64)
                sc.activation(r[:, :], r[:, :], AF.Exp, scale=-0.5)
                st[("r", k)] = r

            LVL = {"p2a": 0, "p2b": 1, "p2e": 1, "p2c": 2,
                   "p2d": 3}.get(only, 9)

            def stage_b(k, only=only):
                if LVL < 1:
                    return
                yt2 = yt_st.ap().rearrange("p r c -> p (r c)")
                r_b = st.pop(("r", k))
                for i in range(GB // 2):
                    c0 = (k * 8 + i) * 128
                    te.matmul(o2[:, i * 64:(i + 1) * 64],
                              yt2[:, c0:c0 + 128], woutbd_s[:, :])
                if LVL < 2:
                    return
                on = onp.tile([128, GB, D_MODEL], F32)
                ve.tensor_tensor(
                    out=on[:, :, :],
                    in0=o2.ap().rearrange("p (g c) -> p g c", c=D_MODEL),
                    in1=bcast(r_b[:, :], D_MODEL), op=ALU.mult)
                if LVL < 3:
                    return
                e = ep.tile([128, GB, D_MODEL], F32)
                sc.activation(e[:, :, :], on[:, :, :], AF.Exp)
                se = sep.tile([128, GB], F32)
                ve.tensor_reduce(out=se[:, :], in_=e[:, :, :],
                                 axis=mybir.AxisListType.X, op=ALU.add)
                rec = sep.tile([128, GB], F32)
                ve.reciprocal(rec[:, :], se[:, :])
                if LVL < 9:
                    return
                os_t = osp.tile([128, GB, D_MODEL], F32)
                gp.tensor_tensor(out=os_t[:, :, :], in0=e[:, :, :],
                                 in1=bcast(rec[:, :], D_MODEL), op=ALU.mult)
                sy.dma_start(
                    out_d.ap().rearrange("(nb p) c -> p nb c", p=128)
                    [:, k * GB:(k + 1) * GB, :], os_t[:, :, :])

            if only != "p1":
                for k in range(NM + 1):
                    if k < NM:
                        stage_a(k)
                    if k >= 1:
                        stage_b(k - 1)
            if only is not None:
                z = osp.tile([128, NB, D_MODEL], F32)
                ve.memset(z, 0.0)
                sy.dma_start(
                    out_d.ap().rearrange("(nb p) c -> p nb c", p=128), z)
    return nc


CTRL_OPS = ("Drain", "NoOp", "Nop", "EventSemaphoreOp", "SemaphoreOp")


def split_overloaded_waits(nc, cap=1, ctrl_only=False):
    n_fixed = 0
    for f in nc.m.functions:
        for bb in f.blocks:
            insts = bb.instructions
            i = 0
            while i < len(insts):
                ins = insts[i]
                si = ins.sync_info
                eff_cap = cap
                if ctrl_only and str(ins.opcode) not in CTRL_OPS:
                    eff_cap = 255
                if si is not None and si.on_wait and len(si.on_wait) > eff_cap:
                    waits = list(si.on_wait)
                    extra, keep = waits[:-cap], waits[-cap:]
                    pos = i
                    for j in range(0, len(extra), cap):
                        chunk = extra[j:j + cap]  # noqa
                        nop = mybir.InstNoOp(
                            name=nc.get_next_instruction_name(), ins=[], outs=[])
                        nop.engine = ins.engine
                        nop.sync_info = mybir.SyncInfo(on_wait=chunk,
                                                       on_update=[])
                        nc.register_instruction(nop)
                        insts.insert(pos, nop)
                        pos += 1
                        i += 1
                    si.on_wait = keep
                    ins.sync_info = si
                    n_fixed += 1
                i += 1
    return n_fixed


def make_in_maps(x, f_out_w, f_out_b, in_proj_w, conv_w, conv_b, dt_bias,
                 A_log, D_skip, norm_w, out_proj_w, num_cores=8):
    N = x.shape[0]
    npc = N // num_cores
    w1t, woutt, db = fold_weights(f_out_w, f_out_b, in_proj_w, conv_w,
                                  conv_b, dt_bias, A_log, D_skip, norm_w,
                                  out_proj_w)
    xt = prep_xt(x)
    bf = ml_dtypes.bfloat16
    consts = {
        "w1t": w1t, "woutt": woutt, "db": db,
        "epsb": np.full((128, 1), NORM_EPS, np.float32),
        "woutbd": np.block(
            [[woutt[:64], np.zeros((64, 32))],
             [np.zeros((64, 32)), woutt[64:]]]).astype(bf),
        "ident": np.eye(128, dtype=bf),
    }
    return [dict(consts,
                 xt=np.ascontiguousarray(xt[:, c * npc:(c + 1) * npc]))
            for c in range(num_cores)], npc


def run(x, f_out_w, f_out_b, in_proj_w, conv_w, conv_b, dt_bias, A_log,
        D_skip, norm_w, out_proj_w, num_cores=8, trace=False):
    from concourse.bass_utils import run_bass_kernel_spmd
    N = x.shape[0]
    assert N % (num_cores * 128 * GB) == 0
    in_maps, npc = make_in_maps(x, f_out_w, f_out_b, in_proj_w, conv_w,
                                conv_b, dt_bias, A_log, D_skip, norm_w,
                                out_proj_w, num_cores)
    nc = build_kernel(npc, num_cores=num_cores)
    split_overloaded_waits(nc)
    res = run_bass_kernel_spmd(nc, in_maps, list(range(num_cores)),
                               trace=trace)
    out = np.concatenate([res.results[c]["out"] for c in range(num_cores)],
                         axis=0)
    return out, res


def kernel(x, f_out_w, f_out_b, in_proj_w, conv_w, conv_b, dt_bias, A_log,
           D_skip, norm_w, out_proj_w):
    out, _ = run(x, f_out_w, f_out_b, in_proj_w, conv_w, conv_b, dt_bias,
                 A_log, D_skip, norm_w, out_proj_w, num_cores=8)
    return out.astype(np.float32)
